# revision 1
# baseline (speedup 1.0000x reference)
"""Trainium2 Bass kernel for nn_NoConsolidationModel (scatter_memory).

Math: reference computes, per batch element b with window w = seqs[b, 55:63]
and query q = query_tok[b]:
    h   = relu(concat(embed[q], mean_j embed[w_j]) @ W1.T + b1)
    out = h @ W2.T + b2
Folding embed into layer 1 (linearity):
    E1a  = embed @ W1[:, :64].T          # [66, 64]
    E1bm = (embed @ W1[:, 64:].T) / 8    # [66, 64]
    h_pre = E1a.T @ onehot(q) + E1bm.T @ counts(w) + b1
so the gather+mean+first-layer collapses into two K=66 matmuls against a
per-element one-hot / count matrix (built on host as uint8, cast to fp16
during the DMA load). PE reduces over K in PSUM; no per-element gathers.

Sharding: pure data parallel, batch split across the 8 cores; tables and
weights replicated.
"""

import sys

sys.path.insert(0, "/opt/trn_rl_repo")

import numpy as np

B = 524288
NCORES = 8
V = 66          # VOCAB_SIZE + 2
H = 64          # HIDDEN_DIM
SEQ = 64
MEM = 8
WIN_LO = SEQ - 1 - MEM
WIN_HI = SEQ - 1

N_PER_CORE = B // NCORES

_PROG_CACHE = {}


def _build_program(n, ch, ts):
    """Build + compile the SPMD Bass program for n batch columns per core.

    ch: columns per DMA chunk; ts: matmul slice width (<=512, PSUM bank).
    Requires n % ch == 0 and ch % (2 * ts) == 0.
    """
    import concourse.tile as tile
    from concourse import bacc, mybir

    assert n % ch == 0 and ch % (2 * ts) == 0
    f16 = mybir.dt.float16
    f32 = mybir.dt.float32
    u8 = mybir.dt.uint8
    Relu = mybir.ActivationFunctionType.Relu

    nc = bacc.Bacc("TRN2", target_bir_lowering=False, debug=False,
                   num_devices=NCORES)

    cnt_d = nc.dram_tensor("cnt", [V, n], u8, kind="ExternalInput").ap()
    ohq_d = nc.dram_tensor("ohq", [V, n], u8, kind="ExternalInput").ap()
    e1bm_d = nc.dram_tensor("e1bm", [V, H], f16, kind="ExternalInput").ap()
    e1a_d = nc.dram_tensor("e1a", [V, H], f16, kind="ExternalInput").ap()
    w2t2_d = nc.dram_tensor("w2t2", [2 * H, H], f16, kind="ExternalInput").ap()
    b1_d = nc.dram_tensor("b1x2", [128, 1], f32, kind="ExternalInput").ap()
    b2_d = nc.dram_tensor("b2x2", [128, 1], f32, kind="ExternalInput").ap()
    out_d = nc.dram_tensor("out", [H, n], f32, kind="ExternalOutput").ap()

    with tile.TileContext(nc) as tc:
        with (
            tc.tile_pool(name="const", bufs=1) as cpool,
            tc.tile_pool(name="cnt", bufs=2) as cnt_pool,
            tc.tile_pool(name="ohq", bufs=2) as ohq_pool,
            tc.tile_pool(name="hbuf", bufs=4) as h_pool,
            tc.tile_pool(name="obuf", bufs=2) as out_pool,
            tc.tile_pool(name="ph", bufs=3, space="PSUM") as ph_pool,
            tc.tile_pool(name="pl", bufs=3, space="PSUM") as pl_pool,
        ):
            e1bm_s = cpool.tile([V, H], f16)
            nc.sync.dma_start(e1bm_s[:], e1bm_d[:])
            e1a_s = cpool.tile([V, H], f16)
            nc.sync.dma_start(e1a_s[:], e1a_d[:])
            w2t2_s = cpool.tile([2 * H, H], f16)
            nc.sync.dma_start(w2t2_s[:], w2t2_d[:])
            b1_s = cpool.tile([128, 1], f32)
            nc.sync.dma_start(b1_s[:], b1_d[:])
            b2_s = cpool.tile([128, 1], f32)
            nc.sync.dma_start(b2_s[:], b2_d[:])

            for c in range(n // ch):
                c0 = c * ch
                cnt_t = cnt_pool.tile([V, ch], f16)
                nc.gpsimd.dma_start(cnt_t[:], cnt_d[:, c0:c0 + ch])  # u8->f16
                ohq_t = ohq_pool.tile([V, ch], f16)
                nc.gpsimd.dma_start(ohq_t[:], ohq_d[:, c0:c0 + ch])
                out_t = out_pool.tile([128, ch // 2], f32)

                for p in range(ch // (2 * ts)):
                    lo = 2 * p * ts          # even slice cols within chunk
                    hi = lo + ts             # odd slice cols
                    ph = ph_pool.tile([128, ts], f32)
                    nc.tensor.matmul(ph[0:H, :], e1bm_s[:], cnt_t[:, lo:hi],
                                     start=True, stop=False)
                    nc.tensor.matmul(ph[0:H, :], e1a_s[:], ohq_t[:, lo:hi],
                                     start=False, stop=True)
                    nc.tensor.matmul(ph[H:128, :], e1bm_s[:],
                                     cnt_t[:, hi:hi + ts],
                                     start=True, stop=False)
                    nc.tensor.matmul(ph[H:128, :], e1a_s[:],
                                     ohq_t[:, hi:hi + ts],
                                     start=False, stop=True)
                    h_t = h_pool.tile([128, ts], f16)
                    nc.scalar.activation(h_t[:], ph[:], Relu, bias=b1_s[:])
                    pl = pl_pool.tile([128, ts], f32)
                    nc.tensor.matmul(pl[0:H, :], w2t2_s[0:H, :], h_t[0:H, :],
                                     start=True, stop=True)
                    nc.tensor.matmul(pl[H:128, :], w2t2_s[H:128, :],
                                     h_t[H:128, :], start=True, stop=True)
                    nc.vector.tensor_scalar_add(
                        out_t[:, p * ts:(p + 1) * ts], pl[:], b2_s[:])

                # out_t[0:64, 512p + j] -> out cols c0 + 1024p + j
                # out_t[64:128, 512p + j] -> out cols c0 + 1024p + 512 + j
                view = out_d[:, c0:c0 + ch].rearrange(
                    "r (pb two j) -> two r pb j", two=2, j=ts)
                nc.sync.dma_start(view[0], out_t[0:H, :])
                nc.sync.dma_start(view[1], out_t[H:128, :])

    nc.compile()
    return nc


def _get_program(n, ch, ts):
    key = (n, ch, ts)
    if key not in _PROG_CACHE:
        _PROG_CACHE[key] = _build_program(n, ch, ts)
    return _PROG_CACHE[key]


def _host_prep(seqs, query_tok, embed, W1, b1, W2, b2, n_cores, n):
    embed = np.asarray(embed, dtype=np.float32)
    W1 = np.asarray(W1, dtype=np.float32)
    W2 = np.asarray(W2, dtype=np.float32)
    b1 = np.asarray(b1, dtype=np.float32)
    b2 = np.asarray(b2, dtype=np.float32)

    e1a = (embed @ W1[:, :H].T).astype(np.float16)            # [V, H]
    e1bm = ((embed @ W1[:, H:].T) / MEM).astype(np.float16)   # [V, H]
    w2t2 = np.concatenate([W2.T, W2.T], axis=0).astype(np.float16)
    b1x2 = np.concatenate([b1, b1]).reshape(128, 1).astype(np.float32)
    b2x2 = np.concatenate([b2, b2]).reshape(128, 1).astype(np.float32)

    win = np.ascontiguousarray(np.asarray(seqs)[:, WIN_LO:WIN_HI]).astype(
        np.int64, copy=False)                                  # [B', MEM]
    q = np.asarray(query_tok).astype(np.int64, copy=False)

    cols = np.arange(n, dtype=np.int64)
    in_maps = []
    for c in range(n_cores):
        w_c = win[c * n:(c + 1) * n]
        q_c = q[c * n:(c + 1) * n]
        flat = w_c * n + cols[:, None]
        cnt = np.bincount(flat.ravel(), minlength=V * n)
        cnt = cnt.astype(np.uint8).reshape(V, n)
        ohq = np.zeros((V, n), dtype=np.uint8)
        ohq[q_c, cols] = 1
        in_maps.append({
            "cnt": cnt, "ohq": ohq, "e1bm": e1bm, "e1a": e1a,
            "w2t2": w2t2, "b1x2": b1x2, "b2x2": b2x2,
        })
    return in_maps


def kernel(seqs, query_tok, embed, W1, b1, W2, b2):
    from concourse.bass_utils import run_bass_kernel_spmd

    n = N_PER_CORE
    in_maps = _host_prep(seqs, query_tok, embed, W1, b1, W2, b2, NCORES, n)
    nc = _get_program(n, ch=8192, ts=512)
    res = run_bass_kernel_spmd(nc, in_maps, core_ids=list(range(NCORES)))
    out = np.empty((B, H), dtype=np.float32)
    for c in range(NCORES):
        out[c * n:(c + 1) * n] = res.results[c]["out"].T
    return out


# revision 2
# speedup vs baseline: 1.5741x; 1.5741x over previous
"""Trainium2 Bass kernel for nn_NoConsolidationModel (scatter_memory).

Math: per batch element with window w = seqs[b, 55:63], query q:
    h   = relu(concat(embed[q], mean_j embed[w_j]) @ W1.T + b1)
    out = h @ W2.T + b2
Folding embed into layer 1 (linearity):
    E1a  = embed @ W1[:, :64].T          # [66, 64]
    E1bm = (embed @ W1[:, 64:].T) / 8    # [66, 64]
    h_pre = E1a.T @ onehot(q) + E1bm.T @ counts(w) + b1
so gather+mean+layer1 collapse into matmuls against count/one-hot vectors
(exact small ints, shipped as fp8e4m3; tables fp16 — PE accepts mixed).

Layout tricks (PE on this part is capped at 1 out-column / 1.2GHz-cycle):
  - single-pass L1: u = [counts(66); onehot_q(62)] has K=128, valid when
    q < 62.  Each core's batch is permuted so q>=62 elements sit in the
    tail; a second small loop recomputes the last TAIL columns with the
    classic two-matmul (counts, full one-hot) form that works for any q.
  - even/odd 512-col slices share one [128, 512] PSUM tile (partitions
    0-63 / 64-127) so ACT/DVE run at full width.
  - block-diagonal L2: lhsT = [[W2.T, 0], [0, W2.T]] computes both
    packed slices' logits in ONE matmul (two elements per PE column).

Sharding: pure data parallel, batch split across 8 cores; weights
replicated. Output stored as [64, n] f16 per core, transposed + upcast on
host.
"""

import sys

sys.path.insert(0, "/opt/trn_rl_repo")

import numpy as np
import ml_dtypes

B = 524288
NCORES = 8
V = 66          # VOCAB_SIZE + 2
H = 64          # HIDDEN_DIM
SEQ = 64
MEM = 8
WIN_LO = SEQ - 1 - MEM
WIN_HI = SEQ - 1
QSPLIT = 128 - V  # 62: queries below this go through the single-pass path

N_PER_CORE = B // NCORES
CH = 8192       # columns per DMA chunk
TS = 512        # matmul slice width (one PSUM bank of f32)
TAIL = 8192     # tail columns recomputed by the any-q path

F8 = ml_dtypes.float8_e4m3

_PROG_CACHE = {}


def _build_program(n, ch, ts, tail):
    import concourse.tile as tile
    from concourse import bacc, mybir

    assert n % ch == 0 and ch % (2 * ts) == 0 and tail % (2 * ts) == 0
    f16 = mybir.dt.float16
    f32 = mybir.dt.float32
    f8 = mybir.dt.float8e4
    Relu = mybir.ActivationFunctionType.Relu

    nc = bacc.Bacc("TRN2", target_bir_lowering=False, debug=False,
                   num_devices=NCORES)

    u1_d = nc.dram_tensor("u1", [128, n], f8, kind="ExternalInput").ap()
    ohq2_d = nc.dram_tensor("ohq2", [V, tail], f8, kind="ExternalInput").ap()
    t1_d = nc.dram_tensor("t1", [128, H], f16, kind="ExternalInput").ap()
    e1bm_d = nc.dram_tensor("e1bm", [V, H], f16, kind="ExternalInput").ap()
    e1a_d = nc.dram_tensor("e1a", [V, H], f16, kind="ExternalInput").ap()
    w2b_d = nc.dram_tensor("w2b", [128, 128], f16, kind="ExternalInput").ap()
    b1_d = nc.dram_tensor("b1x2", [128, 1], f32, kind="ExternalInput").ap()
    b2_d = nc.dram_tensor("b2x2", [128, 1], f32, kind="ExternalInput").ap()
    out1_d = nc.dram_tensor("out1", [H, n], f16, kind="ExternalOutput").ap()
    out2_d = nc.dram_tensor("out2", [H, tail], f16, kind="ExternalOutput").ap()

    with tile.TileContext(nc) as tc:
        with (
            tc.tile_pool(name="const", bufs=1) as cpool,
            tc.tile_pool(name="uin", bufs=3) as u_pool,
            tc.tile_pool(name="hbuf", bufs=4) as h_pool,
            tc.tile_pool(name="obuf", bufs=2) as out_pool,
            tc.tile_pool(name="ph", bufs=3, space="PSUM") as ph_pool,
            tc.tile_pool(name="pl", bufs=3, space="PSUM") as pl_pool,
        ):
            t1_s = cpool.tile([128, H], f16)
            nc.sync.dma_start(t1_s[:], t1_d[:])
            e1bm_s = cpool.tile([V, H], f16)
            nc.sync.dma_start(e1bm_s[:], e1bm_d[:])
            e1a_s = cpool.tile([V, H], f16)
            nc.sync.dma_start(e1a_s[:], e1a_d[:])
            w2b_s = cpool.tile([128, 128], f16)
            nc.sync.dma_start(w2b_s[:], w2b_d[:])
            b1_s = cpool.tile([128, 1], f32)
            nc.sync.dma_start(b1_s[:], b1_d[:])
            b2_s = cpool.tile([128, 1], f32)
            nc.sync.dma_start(b2_s[:], b2_d[:])

            def tail_pair(ph, cnt_t, ohq_t, lo, hi):
                # two-matmul L1 for any q: counts then full one-hot query
                nc.tensor.matmul(ph[0:H, :], e1bm_s[:], cnt_t[:, lo:hi],
                                 start=True, stop=False)
                nc.tensor.matmul(ph[0:H, :], e1a_s[:], ohq_t[:, lo:hi],
                                 start=False, stop=True)
                nc.tensor.matmul(ph[H:128, :], e1bm_s[:], cnt_t[:, hi:hi + ts],
                                 start=True, stop=False)
                nc.tensor.matmul(ph[H:128, :], e1a_s[:], ohq_t[:, hi:hi + ts],
                                 start=False, stop=True)

            def body(ph, out_t, p, l1_emit):
                h_t = h_pool.tile([128, ts], f16)
                nc.scalar.activation(h_t[:], ph[:], Relu, bias=b1_s[:])
                pl = pl_pool.tile([128, ts], f32)
                nc.tensor.matmul(pl[:], w2b_s[:], h_t[:], start=True, stop=True)
                nc.vector.tensor_scalar_add(
                    out_t[:, p * ts:(p + 1) * ts], pl[:], b2_s[:])

            def store(out_dram, c0, chunk, out_t):
                view = out_dram[:, c0:c0 + chunk].rearrange(
                    "r (pb two j) -> two r pb j", two=2, j=ts)
                nc.sync.dma_start(view[0], out_t[0:H, :])
                nc.sync.dma_start(view[1], out_t[H:128, :])

            # region 1: single-pass L1, all n columns
            for c in range(n // ch):
                c0 = c * ch
                u_t = u_pool.tile([128, ch], f8, tag="u1")
                nc.scalar.dma_start(u_t[:], u1_d[:, c0:c0 + ch])
                out_t = out_pool.tile([128, ch // 2], f16, tag="o")
                for p in range(ch // (2 * ts)):
                    lo = 2 * p * ts
                    hi = lo + ts
                    ph = ph_pool.tile([128, ts], f32)
                    nc.tensor.matmul(ph[0:H, :], t1_s[:], u_t[:, lo:hi],
                                     start=True, stop=True)
                    nc.tensor.matmul(ph[H:128, :], t1_s[:], u_t[:, hi:hi + ts],
                                     start=True, stop=True)
                    body(ph, out_t, p, None)
                store(out1_d, c0, ch, out_t)

            # region 2: recompute the tail with the any-q two-pass form
            cnt_t = u_pool.tile([V, tail], f8, tag="cnt2")
            nc.scalar.dma_start(cnt_t[:], u1_d[0:V, n - tail:n])
            ohq_t = u_pool.tile([V, tail], f8, tag="ohq2")
            nc.scalar.dma_start(ohq_t[:], ohq2_d[:])
            out_t2 = out_pool.tile([128, tail // 2], f16, tag="o2")
            for p in range(tail // (2 * ts)):
                lo = 2 * p * ts
                hi = lo + ts
                ph = ph_pool.tile([128, ts], f32)
                tail_pair(ph, cnt_t, ohq_t, lo, hi)
                body(ph, out_t2, p, None)
            store(out2_d, 0, tail, out_t2)

    nc.compile()
    return nc


def _get_program(n, ch, ts, tail):
    key = (n, ch, ts, tail)
    if key not in _PROG_CACHE:
        _PROG_CACHE[key] = _build_program(n, ch, ts, tail)
    return _PROG_CACHE[key]


def _host_prep(seqs, query_tok, embed, W1, b1, W2, b2, n_cores, n, tail):
    embed = np.asarray(embed, dtype=np.float32)
    W1 = np.asarray(W1, dtype=np.float32)
    W2 = np.asarray(W2, dtype=np.float32)
    b1 = np.asarray(b1, dtype=np.float32)
    b2 = np.asarray(b2, dtype=np.float32)

    e1a = (embed @ W1[:, :H].T).astype(np.float16)            # [V, H]
    e1bm = ((embed @ W1[:, H:].T) / MEM).astype(np.float16)   # [V, H]
    t1 = np.concatenate([e1bm, e1a[:QSPLIT]], axis=0)         # [128, H]
    w2b = np.zeros((128, 128), dtype=np.float16)
    w2t = W2.T.astype(np.float16)
    w2b[:H, :H] = w2t
    w2b[H:, H:] = w2t
    b1x2 = np.concatenate([b1, b1]).reshape(128, 1).astype(np.float32)
    b2x2 = np.concatenate([b2, b2]).reshape(128, 1).astype(np.float32)

    win = np.ascontiguousarray(np.asarray(seqs)[:, WIN_LO:WIN_HI]).astype(
        np.int64, copy=False)                                  # [B', MEM]
    q = np.asarray(query_tok).astype(np.int64, copy=False)

    cols = np.arange(n, dtype=np.int64)
    in_maps = []
    perms = []
    for c in range(n_cores):
        w_c = win[c * n:(c + 1) * n]
        q_c = q[c * n:(c + 1) * n]
        hi_q = q_c >= QSPLIT
        n2 = int(hi_q.sum())
        assert n2 <= tail, f"core {c}: {n2} high-query elements > tail {tail}"
        perm = np.concatenate([np.flatnonzero(~hi_q), np.flatnonzero(hi_q)])
        perms.append(perm)
        wp = w_c[perm]
        qp = q_c[perm]
        u1 = np.zeros((128, n), dtype=np.uint8)
        flat = wp * n + cols[:, None]
        u1[:V] = np.bincount(flat.ravel(), minlength=V * n).astype(
            np.uint8).reshape(V, n)
        low = np.flatnonzero(qp < QSPLIT)
        u1[V + qp[low], low] = 1
        ohq2 = np.zeros((V, tail), dtype=np.uint8)
        ohq2[qp[n - tail:], np.arange(tail)] = 1
        in_maps.append({
            "u1": u1.astype(F8), "ohq2": ohq2.astype(F8),
            "t1": t1, "e1bm": e1bm, "e1a": e1a, "w2b": w2b,
            "b1x2": b1x2, "b2x2": b2x2,
        })
    return in_maps, perms


def _assemble(results, perms, n, tail):
    out = np.empty((len(perms) * n, H), dtype=np.float32)
    for c, perm in enumerate(perms):
        o1 = results[c]["out1"].astype(np.float32).T      # [n, H] permuted
        o2 = results[c]["out2"].astype(np.float32).T      # [tail, H]
        o1[n - tail:] = o2
        out[c * n:(c + 1) * n][perm] = o1
    return out


def kernel(seqs, query_tok, embed, W1, b1, W2, b2):
    from concourse.bass_utils import run_bass_kernel_spmd

    n = N_PER_CORE
    in_maps, perms = _host_prep(seqs, query_tok, embed, W1, b1, W2, b2,
                                NCORES, n, TAIL)
    nc = _get_program(n, CH, TS, TAIL)
    res = run_bass_kernel_spmd(nc, in_maps, core_ids=list(range(NCORES)))
    return _assemble(res.results, perms, n, TAIL)


# revision 5
# speedup vs baseline: 2.2390x; 1.4224x over previous
"""Trainium2 Bass kernel for nn_NoConsolidationModel (scatter_memory).

Math: per batch element with window w = seqs[b, 55:63], query q:
    h   = relu(concat(embed[q], mean_j embed[w_j]) @ W1.T + b1)
    out = h @ W2.T + b2
Folding embed into layer 1 (linearity):
    E1a  = embed @ W1[:, :64].T          # [66, 64]
    E1bm = (embed @ W1[:, 64:].T) / 8    # [66, 64]
    h_pre = E1a.T @ onehot(q) + E1bm.T @ counts(w) + b1
so gather+mean+layer1 collapse into matmuls against count/one-hot vectors
(exact small ints, shipped as fp8e4m3; tables fp16 — PE accepts mixed).

Layout tricks (PE here is capped at 1 out-column per 1.2GHz cycle):
  - single-pass L1: u = [counts(66); onehot_q(62)] has K=128, valid when
    q < 62.  Each core's batch is permuted so q>=62 elements sit in the
    tail; a small second loop recomputes the last TAIL columns with the
    classic two-matmul (counts, full one-hot) form that works for any q.
  - even/odd 512-col slices share one [128, *] PSUM tile (partitions
    0-63 / 64-127) so ACT/DVE run at full width; two pairs share one
    [128, 1024] PSUM tile so ACT/DVE run one op per 2048 elements.
  - block-diagonal L2: lhsT = [[W2.T, 0], [0, W2.T]] computes both
    packed slices' logits in ONE matmul (two elements per PE column).

Sharding: pure data parallel, batch split across 8 cores; weights
replicated. Output stored as [64, n] f16 per core, transposed + upcast on
host.
"""

import sys

sys.path.insert(0, "/opt/trn_rl_repo")

import numpy as np
import ml_dtypes

B = 524288
NCORES = 8
V = 66          # VOCAB_SIZE + 2
H = 64          # HIDDEN_DIM
SEQ = 64
MEM = 8
WIN_LO = SEQ - 1 - MEM
WIN_HI = SEQ - 1
QSPLIT = 128 - V  # 62: queries below this go through the single-pass path

N_PER_CORE = B // NCORES
CH = 4096       # columns per DMA chunk (4 pairs = 2 groups)
TS = 512        # matmul slice width (one PSUM bank of f32)
TAIL = 5120     # tail columns recomputed by the any-q path

F8 = ml_dtypes.float8_e4m3

_PROG_CACHE = {}


def _build_program(n, ch, ts, tail):
    import concourse.tile as tile
    from concourse import bacc, mybir

    assert n % ch == 0 and ch % (4 * ts) == 0 and tail % (2 * ts) == 0
    f16 = mybir.dt.float16
    f32 = mybir.dt.float32
    f8 = mybir.dt.float8e4
    Relu = mybir.ActivationFunctionType.Relu

    nc = bacc.Bacc("TRN2", target_bir_lowering=False, debug=False,
                   num_devices=NCORES)

    u1_d = nc.dram_tensor("u1", [128, n], f8, kind="ExternalInput").ap()
    ohq2_d = nc.dram_tensor("ohq2", [V, tail], f8, kind="ExternalInput").ap()
    t1_d = nc.dram_tensor("t1", [128, H], f16, kind="ExternalInput").ap()
    e1bm_d = nc.dram_tensor("e1bm", [V, H], f16, kind="ExternalInput").ap()
    e1a_d = nc.dram_tensor("e1a", [V, H], f16, kind="ExternalInput").ap()
    w2b_d = nc.dram_tensor("w2b", [128, 128], f16, kind="ExternalInput").ap()
    b1_d = nc.dram_tensor("b1x2", [128, 1], f32, kind="ExternalInput").ap()
    b2_d = nc.dram_tensor("b2x2", [128, 1], f32, kind="ExternalInput").ap()
    out1_d = nc.dram_tensor("out1", [H, n], f16, kind="ExternalOutput").ap()
    out2_d = nc.dram_tensor("out2", [H, tail], f16, kind="ExternalOutput").ap()

    with tile.TileContext(nc) as tc:
        with (
            tc.tile_pool(name="const", bufs=1) as cpool,
            tc.tile_pool(name="uin", bufs=6) as u_pool,
            tc.tile_pool(name="tin", bufs=1) as t_pool,
            tc.tile_pool(name="hbuf", bufs=4) as h_pool,
            tc.tile_pool(name="obuf", bufs=3) as out_pool,
            tc.tile_pool(name="ph", bufs=2, space="PSUM") as ph_pool,
            tc.tile_pool(name="pl", bufs=2, space="PSUM") as pl_pool,
        ):
            # constants via SWDGE: keeps the HWDGE rings free for streaming
            t1_s = cpool.tile([128, H], f16)
            nc.gpsimd.dma_start(t1_s[:], t1_d[:])
            e1bm_s = cpool.tile([V, H], f16)
            nc.gpsimd.dma_start(e1bm_s[:], e1bm_d[:])
            e1a_s = cpool.tile([V, H], f16)
            nc.gpsimd.dma_start(e1a_s[:], e1a_d[:])
            w2b_s = cpool.tile([128, 128], f16)
            nc.gpsimd.dma_start(w2b_s[:], w2b_d[:])
            b1_s = cpool.tile([128, 1], f32)
            nc.gpsimd.dma_start(b1_s[:], b1_d[:])
            b2_s = cpool.tile([128, 1], f32)
            nc.gpsimd.dma_start(b2_s[:], b2_d[:])

            def l2_and_out(ph, out_t, col0, width):
                # relu+bias, block-diag L2, +b2 with PSUM->SBUF f16 copy
                h_t = h_pool.tile([128, 2 * ts], f16, tag="h")
                nc.scalar.activation(h_t[:, :width], ph[:, :width], Relu,
                                     bias=b1_s[:])
                pl = pl_pool.tile([128, 2 * ts], f32, tag="pl")
                for s in range(width // ts):
                    nc.tensor.matmul(pl[:, s * ts:(s + 1) * ts],
                                     w2b_s[:], h_t[:, s * ts:(s + 1) * ts],
                                     start=True, stop=True)
                nc.vector.tensor_scalar_add(
                    out_t[:, col0:col0 + width], pl[:, :width], b2_s[:])

            def store(out_dram, c0, chunk, out_t):
                view = out_dram[:, c0:c0 + chunk].rearrange(
                    "r (pb two j) -> two r pb j", two=2, j=ts)
                nc.sync.dma_start(view[0], out_t[0:H, :])
                nc.sync.dma_start(view[1], out_t[H:128, :])

            # region 1: single-pass L1 (K=128), all n columns
            for c in range(n // ch):
                c0 = c * ch
                u_t = u_pool.tile([128, ch], f8, tag="u1")
                nc.scalar.dma_start(u_t[:], u1_d[:, c0:c0 + ch])
                out_t = out_pool.tile([128, ch // 2], f16, tag="o")
                for g in range(ch // (4 * ts)):   # group: 2 pairs = 4 slices
                    lo = g * 4 * ts
                    ph = ph_pool.tile([128, 2 * ts], f32, tag="ph")
                    for half in range(2):         # pair within group
                        a = lo + 2 * half * ts
                        po = half * ts
                        nc.tensor.matmul(ph[0:H, po:po + ts], t1_s[:],
                                         u_t[:, a:a + ts],
                                         start=True, stop=True)
                        nc.tensor.matmul(ph[H:128, po:po + ts], t1_s[:],
                                         u_t[:, a + ts:a + 2 * ts],
                                         start=True, stop=True)
                    l2_and_out(ph, out_t, g * 2 * ts, 2 * ts)
                store(out1_d, c0, ch, out_t)

            # region 2: recompute the tail with the any-q two-pass form
            cnt_t = t_pool.tile([V, tail], f8, tag="cnt2")
            nc.scalar.dma_start(cnt_t[:], u1_d[0:V, n - tail:n])
            ohq_t = t_pool.tile([V, tail], f8, tag="ohq2")
            nc.scalar.dma_start(ohq_t[:], ohq2_d[:])
            out_t2 = out_pool.tile([128, tail // 2], f16, tag="o2")
            for p in range(tail // (2 * ts)):
                lo = 2 * p * ts
                hi = lo + ts
                ph = ph_pool.tile([128, 2 * ts], f32, tag="ph")
                for col, a in ((slice(0, H), lo), (slice(H, 128), hi)):
                    nc.tensor.matmul(ph[col, 0:ts], e1bm_s[:],
                                     cnt_t[:, a:a + ts],
                                     start=True, stop=False)
                    nc.tensor.matmul(ph[col, 0:ts], e1a_s[:],
                                     ohq_t[:, a:a + ts],
                                     start=False, stop=True)
                l2_and_out(ph, out_t2, p * ts, ts)
            store(out2_d, 0, tail, out_t2)

    nc.compile()
    return nc


def _get_program(n, ch, ts, tail):
    key = (n, ch, ts, tail)
    if key not in _PROG_CACHE:
        _PROG_CACHE[key] = _build_program(n, ch, ts, tail)
    return _PROG_CACHE[key]


def _host_prep(seqs, query_tok, embed, W1, b1, W2, b2, n_cores, n, tail):
    embed = np.asarray(embed, dtype=np.float32)
    W1 = np.asarray(W1, dtype=np.float32)
    W2 = np.asarray(W2, dtype=np.float32)
    b1 = np.asarray(b1, dtype=np.float32)
    b2 = np.asarray(b2, dtype=np.float32)

    e1a = (embed @ W1[:, :H].T).astype(np.float16)            # [V, H]
    e1bm = ((embed @ W1[:, H:].T) / MEM).astype(np.float16)   # [V, H]
    t1 = np.concatenate([e1bm, e1a[:QSPLIT]], axis=0)         # [128, H]
    w2b = np.zeros((128, 128), dtype=np.float16)
    w2t = W2.T.astype(np.float16)
    w2b[:H, :H] = w2t
    w2b[H:, H:] = w2t
    b1x2 = np.concatenate([b1, b1]).reshape(128, 1).astype(np.float32)
    b2x2 = np.concatenate([b2, b2]).reshape(128, 1).astype(np.float32)

    win = np.ascontiguousarray(np.asarray(seqs)[:, WIN_LO:WIN_HI]).astype(
        np.int64, copy=False)                                  # [B', MEM]
    q = np.asarray(query_tok).astype(np.int64, copy=False)

    cols = np.arange(n, dtype=np.int64)
    in_maps = []
    perms = []
    for c in range(n_cores):
        w_c = win[c * n:(c + 1) * n]
        q_c = q[c * n:(c + 1) * n]
        hi_q = q_c >= QSPLIT
        n2 = int(hi_q.sum())
        assert n2 <= tail, f"core {c}: {n2} high-query elements > tail {tail}"
        perm = np.concatenate([np.flatnonzero(~hi_q), np.flatnonzero(hi_q)])
        perms.append(perm)
        wp = w_c[perm]
        qp = q_c[perm]
        u1 = np.zeros((128, n), dtype=np.uint8)
        flat = wp * n + cols[:, None]
        u1[:V] = np.bincount(flat.ravel(), minlength=V * n).astype(
            np.uint8).reshape(V, n)
        low = np.flatnonzero(qp < QSPLIT)
        u1[V + qp[low], low] = 1
        ohq2 = np.zeros((V, tail), dtype=np.uint8)
        ohq2[qp[n - tail:], np.arange(tail)] = 1
        in_maps.append({
            "u1": u1.astype(F8), "ohq2": ohq2.astype(F8),
            "t1": t1, "e1bm": e1bm, "e1a": e1a, "w2b": w2b,
            "b1x2": b1x2, "b2x2": b2x2,
        })
    return in_maps, perms


def _assemble(results, perms, n, tail):
    out = np.empty((len(perms) * n, H), dtype=np.float32)
    for c, perm in enumerate(perms):
        o1 = results[c]["out1"].astype(np.float32).T      # [n, H] permuted
        o2 = results[c]["out2"].astype(np.float32).T      # [tail, H]
        o1[n - tail:] = o2
        out[c * n:(c + 1) * n][perm] = o1
    return out


def kernel(seqs, query_tok, embed, W1, b1, W2, b2):
    from concourse.bass_utils import run_bass_kernel_spmd

    n = N_PER_CORE
    in_maps, perms = _host_prep(seqs, query_tok, embed, W1, b1, W2, b2,
                                NCORES, n, TAIL)
    nc = _get_program(n, CH, TS, TAIL)
    res = run_bass_kernel_spmd(nc, in_maps, core_ids=list(range(NCORES)))
    return _assemble(res.results, perms, n, TAIL)


# revision 6
# speedup vs baseline: 2.2679x; 1.0129x over previous
"""Trainium2 Bass kernel for nn_NoConsolidationModel (scatter_memory).

Math: per batch element with window w = seqs[b, 55:63], query q:
    h   = relu(concat(embed[q], mean_j embed[w_j]) @ W1.T + b1)
    out = h @ W2.T + b2
Folding embed into layer 1 (linearity):
    E1a  = embed @ W1[:, :64].T          # [66, 64]
    E1bm = (embed @ W1[:, 64:].T) / 8    # [66, 64]
    h_pre = E1a.T @ onehot(q) + E1bm.T @ counts(w) + b1
so gather+mean+layer1 collapse into matmuls against count/one-hot vectors
(exact small ints, shipped as fp8e4m3; tables fp16 — PE accepts mixed).

Layout tricks (PE here is capped at 1 out-column per 1.2GHz cycle):
  - single-pass L1: u = [counts(66); onehot_q(62)] has K=128, valid when
    q < 62.  Each core's batch is permuted so q>=62 elements sit in the
    tail; a small second loop recomputes the last TAIL columns with the
    classic two-matmul (counts, full one-hot) form that works for any q.
  - even/odd 512-col slices share one [128, *] PSUM tile (partitions
    0-63 / 64-127) so ACT/DVE run at full width; two pairs share one
    [128, 1024] PSUM tile so ACT/DVE run one op per 2048 elements.
  - block-diagonal L2: lhsT = [[W2.T, 0], [0, W2.T]] computes both
    packed slices' logits in ONE matmul (two elements per PE column).

Sharding: pure data parallel, batch split across 8 cores; weights
replicated. Output stored as [64, n] f16 per core, transposed + upcast on
host.
"""

import sys

sys.path.insert(0, "/opt/trn_rl_repo")

import numpy as np
import ml_dtypes

B = 524288
NCORES = 8
V = 66          # VOCAB_SIZE + 2
H = 64          # HIDDEN_DIM
SEQ = 64
MEM = 8
WIN_LO = SEQ - 1 - MEM
WIN_HI = SEQ - 1
QSPLIT = 128 - V  # 62: queries below this go through the single-pass path

N_PER_CORE = B // NCORES
CH = 4096       # columns per DMA chunk (4 pairs = 2 groups)
TS = 512        # matmul slice width (one PSUM bank of f32)
TAIL = 5120     # tail columns recomputed by the any-q path

F8 = ml_dtypes.float8_e4m3

_PROG_CACHE = {}


def _build_program(n, ch, ts, tail):
    import concourse.tile as tile
    from concourse import bacc, mybir

    assert n % ch == 0 and ch % (4 * ts) == 0 and tail % (2 * ts) == 0
    f16 = mybir.dt.float16
    f32 = mybir.dt.float32
    f8 = mybir.dt.float8e4
    Relu = mybir.ActivationFunctionType.Relu

    nc = bacc.Bacc("TRN2", target_bir_lowering=False, debug=False,
                   num_devices=NCORES)

    u1_d = nc.dram_tensor("u1", [128, n], f8, kind="ExternalInput").ap()
    ohq2_d = nc.dram_tensor("ohq2", [V, tail], f8, kind="ExternalInput").ap()
    t1_d = nc.dram_tensor("t1", [128, H], f16, kind="ExternalInput").ap()
    e1bm_d = nc.dram_tensor("e1bm", [V, H], f16, kind="ExternalInput").ap()
    e1a_d = nc.dram_tensor("e1a", [V, H], f16, kind="ExternalInput").ap()
    w2b_d = nc.dram_tensor("w2b", [128, 128], f16, kind="ExternalInput").ap()
    b1_d = nc.dram_tensor("b1x2", [128, 1], f32, kind="ExternalInput").ap()
    b2_d = nc.dram_tensor("b2x2", [128, 1], f32, kind="ExternalInput").ap()
    out1_d = nc.dram_tensor("out1", [H, n], f16, kind="ExternalOutput").ap()
    out2_d = nc.dram_tensor("out2", [H, tail], f16, kind="ExternalOutput").ap()

    with tile.TileContext(nc) as tc:
        with (
            tc.tile_pool(name="const", bufs=1) as cpool,
            tc.tile_pool(name="uin", bufs=6) as u_pool,
            tc.tile_pool(name="tin", bufs=1) as t_pool,
            tc.tile_pool(name="hbuf", bufs=6) as h_pool,
            tc.tile_pool(name="obuf", bufs=4) as out_pool,
            tc.tile_pool(name="ph", bufs=2, space="PSUM") as ph_pool,
            tc.tile_pool(name="pl", bufs=2, space="PSUM") as pl_pool,
        ):
            # constants on the sync HWDGE ring (scalar ring streams u1)
            t1_s = cpool.tile([128, H], f16)
            nc.sync.dma_start(t1_s[:], t1_d[:])
            w2b_s = cpool.tile([128, 128], f16)
            nc.sync.dma_start(w2b_s[:], w2b_d[:])
            b1_s = cpool.tile([128, 1], f32)
            nc.sync.dma_start(b1_s[:], b1_d[:])
            b2_s = cpool.tile([128, 1], f32)
            nc.sync.dma_start(b2_s[:], b2_d[:])
            e1bm_s = cpool.tile([V, H], f16)
            nc.sync.dma_start(e1bm_s[:], e1bm_d[:])
            e1a_s = cpool.tile([V, H], f16)
            nc.sync.dma_start(e1a_s[:], e1a_d[:])

            def l2_and_out(ph, out_t, col0, width):
                # relu+bias, block-diag L2, +b2 with PSUM->SBUF f16 copy
                h_t = h_pool.tile([128, 2 * ts], f16, tag="h")
                nc.scalar.activation(h_t[:, :width], ph[:, :width], Relu,
                                     bias=b1_s[:])
                pl = pl_pool.tile([128, 2 * ts], f32, tag="pl")
                for s in range(width // ts):
                    nc.tensor.matmul(pl[:, s * ts:(s + 1) * ts],
                                     w2b_s[:], h_t[:, s * ts:(s + 1) * ts],
                                     start=True, stop=True)
                nc.vector.tensor_scalar_add(
                    out_t[:, col0:col0 + width], pl[:, :width], b2_s[:])

            def store(out_dram, c0, chunk, out_t):
                view = out_dram[:, c0:c0 + chunk].rearrange(
                    "r (pb two j) -> two r pb j", two=2, j=ts)
                nc.sync.dma_start(view[0], out_t[0:H, :])
                nc.sync.dma_start(view[1], out_t[H:128, :])

            # region 1: single-pass L1 (K=128), all n columns
            for c in range(n // ch):
                c0 = c * ch
                u_t = u_pool.tile([128, ch], f8, tag="u1")
                nc.scalar.dma_start(u_t[:], u1_d[:, c0:c0 + ch])
                out_t = out_pool.tile([128, ch // 2], f16, tag="o")
                for g in range(ch // (4 * ts)):   # group: 2 pairs = 4 slices
                    lo = g * 4 * ts
                    ph = ph_pool.tile([128, 2 * ts], f32, tag="ph")
                    for half in range(2):         # pair within group
                        a = lo + 2 * half * ts
                        po = half * ts
                        nc.tensor.matmul(ph[0:H, po:po + ts], t1_s[:],
                                         u_t[:, a:a + ts],
                                         start=True, stop=True)
                        nc.tensor.matmul(ph[H:128, po:po + ts], t1_s[:],
                                         u_t[:, a + ts:a + 2 * ts],
                                         start=True, stop=True)
                    l2_and_out(ph, out_t, g * 2 * ts, 2 * ts)
                store(out1_d, c0, ch, out_t)

            # region 2: recompute the tail with the any-q two-pass form
            cnt_t = t_pool.tile([V, tail], f8, tag="cnt2")
            nc.scalar.dma_start(cnt_t[:], u1_d[0:V, n - tail:n])
            ohq_t = t_pool.tile([V, tail], f8, tag="ohq2")
            nc.scalar.dma_start(ohq_t[:], ohq2_d[:])
            out_t2 = out_pool.tile([128, tail // 2], f16, tag="o2")
            for p in range(tail // (2 * ts)):
                lo = 2 * p * ts
                hi = lo + ts
                ph = ph_pool.tile([128, 2 * ts], f32, tag="ph")
                for col, a in ((slice(0, H), lo), (slice(H, 128), hi)):
                    nc.tensor.matmul(ph[col, 0:ts], e1bm_s[:],
                                     cnt_t[:, a:a + ts],
                                     start=True, stop=False)
                    nc.tensor.matmul(ph[col, 0:ts], e1a_s[:],
                                     ohq_t[:, a:a + ts],
                                     start=False, stop=True)
                l2_and_out(ph, out_t2, p * ts, ts)
            store(out2_d, 0, tail, out_t2)

    nc.compile()
    return nc


def _get_program(n, ch, ts, tail):
    key = (n, ch, ts, tail)
    if key not in _PROG_CACHE:
        _PROG_CACHE[key] = _build_program(n, ch, ts, tail)
    return _PROG_CACHE[key]


def _host_prep(seqs, query_tok, embed, W1, b1, W2, b2, n_cores, n, tail):
    embed = np.asarray(embed, dtype=np.float32)
    W1 = np.asarray(W1, dtype=np.float32)
    W2 = np.asarray(W2, dtype=np.float32)
    b1 = np.asarray(b1, dtype=np.float32)
    b2 = np.asarray(b2, dtype=np.float32)

    e1a = (embed @ W1[:, :H].T).astype(np.float16)            # [V, H]
    e1bm = ((embed @ W1[:, H:].T) / MEM).astype(np.float16)   # [V, H]
    t1 = np.concatenate([e1bm, e1a[:QSPLIT]], axis=0)         # [128, H]
    w2b = np.zeros((128, 128), dtype=np.float16)
    w2t = W2.T.astype(np.float16)
    w2b[:H, :H] = w2t
    w2b[H:, H:] = w2t
    b1x2 = np.concatenate([b1, b1]).reshape(128, 1).astype(np.float32)
    b2x2 = np.concatenate([b2, b2]).reshape(128, 1).astype(np.float32)

    win = np.ascontiguousarray(np.asarray(seqs)[:, WIN_LO:WIN_HI]).astype(
        np.int64, copy=False)                                  # [B', MEM]
    q = np.asarray(query_tok).astype(np.int64, copy=False)

    cols = np.arange(n, dtype=np.int64)
    in_maps = []
    perms = []
    for c in range(n_cores):
        w_c = win[c * n:(c + 1) * n]
        q_c = q[c * n:(c + 1) * n]
        hi_q = q_c >= QSPLIT
        n2 = int(hi_q.sum())
        assert n2 <= tail, f"core {c}: {n2} high-query elements > tail {tail}"
        perm = np.concatenate([np.flatnonzero(~hi_q), np.flatnonzero(hi_q)])
        perms.append(perm)
        wp = w_c[perm]
        qp = q_c[perm]
        u1 = np.zeros((128, n), dtype=np.uint8)
        flat = wp * n + cols[:, None]
        u1[:V] = np.bincount(flat.ravel(), minlength=V * n).astype(
            np.uint8).reshape(V, n)
        low = np.flatnonzero(qp < QSPLIT)
        u1[V + qp[low], low] = 1
        ohq2 = np.zeros((V, tail), dtype=np.uint8)
        ohq2[qp[n - tail:], np.arange(tail)] = 1
        in_maps.append({
            "u1": u1.astype(F8), "ohq2": ohq2.astype(F8),
            "t1": t1, "e1bm": e1bm, "e1a": e1a, "w2b": w2b,
            "b1x2": b1x2, "b2x2": b2x2,
        })
    return in_maps, perms


def _assemble(results, perms, n, tail):
    out = np.empty((len(perms) * n, H), dtype=np.float32)
    for c, perm in enumerate(perms):
        o1 = results[c]["out1"].astype(np.float32).T      # [n, H] permuted
        o2 = results[c]["out2"].astype(np.float32).T      # [tail, H]
        o1[n - tail:] = o2
        out[c * n:(c + 1) * n][perm] = o1
    return out


def kernel(seqs, query_tok, embed, W1, b1, W2, b2):
    from concourse.bass_utils import run_bass_kernel_spmd

    n = N_PER_CORE
    in_maps, perms = _host_prep(seqs, query_tok, embed, W1, b1, W2, b2,
                                NCORES, n, TAIL)
    nc = _get_program(n, CH, TS, TAIL)
    res = run_bass_kernel_spmd(nc, in_maps, core_ids=list(range(NCORES)))
    return _assemble(res.results, perms, n, TAIL)


# revision 7
# speedup vs baseline: 2.3562x; 1.0389x over previous
"""Trainium2 Bass kernel for nn_NoConsolidationModel (scatter_memory).

Math: per batch element with window w = seqs[b, 55:63], query q:
    h   = relu(concat(embed[q], mean_j embed[w_j]) @ W1.T + b1)
    out = h @ W2.T + b2
Folding embed into layer 1 (linearity):
    E1a  = embed @ W1[:, :64].T          # [66, 64]
    E1bm = (embed @ W1[:, 64:].T) / 8    # [66, 64]
    h_pre = E1a.T @ onehot(q) + E1bm.T @ counts(w) + b1
so gather+mean+layer1 collapse into matmuls against count/one-hot vectors
(exact small ints, shipped as fp8e4m3; tables fp16 — PE accepts mixed).

Layout tricks (PE here is capped at 1 out-column per 1.2GHz cycle):
  - single-pass L1: u = [counts(66); onehot_q(62)] has K=128, valid when
    q < 62.  Each core's batch is permuted so q>=62 elements sit in the
    tail; a small second loop recomputes the last TAIL columns with the
    classic two-matmul (counts, full one-hot) form that works for any q.
  - even/odd 512-col slices share one [128, *] PSUM tile (partitions
    0-63 / 64-127) so ACT/DVE run at full width; two pairs share one
    [128, 1024] PSUM tile so ACT/DVE run one op per 2048 elements.
  - block-diagonal L2: lhsT = [[W2.T, 0], [0, W2.T]] computes both
    packed slices' logits in ONE matmul (two elements per PE column).

Sharding: pure data parallel, batch split across 8 cores; weights
replicated. Output stored as [64, n] f16 per core, transposed + upcast on
host.
"""

import sys

sys.path.insert(0, "/opt/trn_rl_repo")

import numpy as np
import ml_dtypes

B = 524288
NCORES = 8
V = 66          # VOCAB_SIZE + 2
H = 64          # HIDDEN_DIM
SEQ = 64
MEM = 8
WIN_LO = SEQ - 1 - MEM
WIN_HI = SEQ - 1
QSPLIT = 128 - V  # 62: queries below this go through the single-pass path

N_PER_CORE = B // NCORES
CH = 4096       # columns per DMA chunk (4 pairs = 2 groups)
TS = 512        # matmul slice width (one PSUM bank of f32)
TAIL = 5120     # tail columns recomputed by the any-q path

F8 = ml_dtypes.float8_e4m3

_PROG_CACHE = {}


def _build_program(n, ch, ts, tail):
    import concourse.tile as tile
    from concourse import bacc, mybir

    assert n % ch == 0 and ch % (4 * ts) == 0 and tail % (2 * ts) == 0
    f16 = mybir.dt.float16
    f32 = mybir.dt.float32
    f8 = mybir.dt.float8e4
    Relu = mybir.ActivationFunctionType.Relu

    nc = bacc.Bacc("TRN2", target_bir_lowering=False, debug=False,
                   num_devices=NCORES)

    u8 = mybir.dt.uint8
    u1_d = nc.dram_tensor("u1", [128, n], f8, kind="ExternalInput").ap()
    ohq2_d = nc.dram_tensor("ohq2", [V, tail], f8, kind="ExternalInput").ap()
    # all weights/biases packed in one blob: [t1 f16x64 | w2b f16x128 |
    # b1 f32 | b2 f32 | e1bm f16x64 | e1a f16x64] per partition row
    cb_d = nc.dram_tensor("cblob", [128, 648], u8, kind="ExternalInput").ap()
    out1_d = nc.dram_tensor("out1", [H, n], f16, kind="ExternalOutput").ap()
    out2_d = nc.dram_tensor("out2", [H, tail], f16, kind="ExternalOutput").ap()

    with tile.TileContext(nc) as tc:
        with (
            tc.tile_pool(name="const", bufs=1) as cpool,
            tc.tile_pool(name="uin", bufs=6) as u_pool,
            tc.tile_pool(name="tin", bufs=1) as t_pool,
            tc.tile_pool(name="hbuf", bufs=6) as h_pool,
            tc.tile_pool(name="obuf", bufs=4) as out_pool,
            tc.tile_pool(name="ph", bufs=2, space="PSUM") as ph_pool,
            tc.tile_pool(name="pl", bufs=2, space="PSUM") as pl_pool,
        ):
            # one DMA for every constant, bitcast views into the blob
            cb_t = cpool.tile([128, 648], u8)
            nc.sync.dma_start(cb_t[:], cb_d[:])
            t1_s = cb_t[:, 0:128].bitcast(f16)          # [128, 64]
            w2b_s = cb_t[:, 128:384].bitcast(f16)       # [128, 128]
            b1_s = cb_t[:, 384:388].bitcast(f32)        # [128, 1]
            b2_s = cb_t[:, 388:392].bitcast(f32)        # [128, 1]
            e1bm_s = cb_t[0:V, 392:520].bitcast(f16)    # [66, 64]
            e1a_s = cb_t[0:V, 520:648].bitcast(f16)     # [66, 64]

            def l2_and_out(ph, out_t, col0, width):
                # relu+bias, block-diag L2, +b2 with PSUM->SBUF f16 copy
                h_t = h_pool.tile([128, 2 * ts], f16, tag="h")
                nc.scalar.activation(h_t[:, :width], ph[:, :width], Relu,
                                     bias=b1_s)
                pl = pl_pool.tile([128, 2 * ts], f32, tag="pl")
                for s in range(width // ts):
                    nc.tensor.matmul(pl[:, s * ts:(s + 1) * ts],
                                     w2b_s, h_t[:, s * ts:(s + 1) * ts],
                                     start=True, stop=True)
                nc.vector.tensor_scalar_add(
                    out_t[:, col0:col0 + width], pl[:, :width], b2_s)

            def store(out_dram, c0, chunk, out_t):
                view = out_dram[:, c0:c0 + chunk].rearrange(
                    "r (pb two j) -> two r pb j", two=2, j=ts)
                nc.sync.dma_start(view[0], out_t[0:H, :])
                nc.sync.dma_start(view[1], out_t[H:128, :])

            # region 1: single-pass L1 (K=128), all n columns
            for c in range(n // ch):
                c0 = c * ch
                u_t = u_pool.tile([128, ch], f8, tag="u1")
                nc.scalar.dma_start(u_t[:], u1_d[:, c0:c0 + ch])
                out_t = out_pool.tile([128, ch // 2], f16, tag="o")
                for g in range(ch // (4 * ts)):   # group: 2 pairs = 4 slices
                    lo = g * 4 * ts
                    ph = ph_pool.tile([128, 2 * ts], f32, tag="ph")
                    for half in range(2):         # pair within group
                        a = lo + 2 * half * ts
                        po = half * ts
                        nc.tensor.matmul(ph[0:H, po:po + ts], t1_s,
                                         u_t[:, a:a + ts],
                                         start=True, stop=True)
                        nc.tensor.matmul(ph[H:128, po:po + ts], t1_s,
                                         u_t[:, a + ts:a + 2 * ts],
                                         start=True, stop=True)
                    l2_and_out(ph, out_t, g * 2 * ts, 2 * ts)
                store(out1_d, c0, ch, out_t)

            # region 2: recompute the tail with the any-q two-pass form
            cnt_t = t_pool.tile([V, tail], f8, tag="cnt2")
            nc.scalar.dma_start(cnt_t[:], u1_d[0:V, n - tail:n])
            ohq_t = t_pool.tile([V, tail], f8, tag="ohq2")
            nc.scalar.dma_start(ohq_t[:], ohq2_d[:])
            out_t2 = out_pool.tile([128, tail // 2], f16, tag="o2")
            for p in range(tail // (2 * ts)):
                lo = 2 * p * ts
                hi = lo + ts
                ph = ph_pool.tile([128, 2 * ts], f32, tag="ph")
                for col, a in ((slice(0, H), lo), (slice(H, 128), hi)):
                    nc.tensor.matmul(ph[col, 0:ts], e1bm_s,
                                     cnt_t[:, a:a + ts],
                                     start=True, stop=False)
                    nc.tensor.matmul(ph[col, 0:ts], e1a_s,
                                     ohq_t[:, a:a + ts],
                                     start=False, stop=True)
                l2_and_out(ph, out_t2, p * ts, ts)
            store(out2_d, 0, tail, out_t2)

    nc.compile()
    return nc


def _get_program(n, ch, ts, tail):
    key = (n, ch, ts, tail)
    if key not in _PROG_CACHE:
        _PROG_CACHE[key] = _build_program(n, ch, ts, tail)
    return _PROG_CACHE[key]


def _host_prep(seqs, query_tok, embed, W1, b1, W2, b2, n_cores, n, tail):
    embed = np.asarray(embed, dtype=np.float32)
    W1 = np.asarray(W1, dtype=np.float32)
    W2 = np.asarray(W2, dtype=np.float32)
    b1 = np.asarray(b1, dtype=np.float32)
    b2 = np.asarray(b2, dtype=np.float32)

    e1a = (embed @ W1[:, :H].T).astype(np.float16)            # [V, H]
    e1bm = ((embed @ W1[:, H:].T) / MEM).astype(np.float16)   # [V, H]
    t1 = np.concatenate([e1bm, e1a[:QSPLIT]], axis=0)         # [128, H]
    w2b = np.zeros((128, 128), dtype=np.float16)
    w2t = W2.T.astype(np.float16)
    w2b[:H, :H] = w2t
    w2b[H:, H:] = w2t
    b1x2 = np.concatenate([b1, b1]).reshape(128, 1).astype(np.float32)
    b2x2 = np.concatenate([b2, b2]).reshape(128, 1).astype(np.float32)
    pad = np.zeros((128 - V, H), dtype=np.float16)
    cblob = np.concatenate([
        t1.view(np.uint8), w2b.view(np.uint8),
        b1x2.view(np.uint8), b2x2.view(np.uint8),
        np.concatenate([e1bm, pad]).view(np.uint8),
        np.concatenate([e1a, pad]).view(np.uint8)], axis=1)   # [128, 648]

    win = np.ascontiguousarray(np.asarray(seqs)[:, WIN_LO:WIN_HI]).astype(
        np.int64, copy=False)                                  # [B', MEM]
    q = np.asarray(query_tok).astype(np.int64, copy=False)

    cols = np.arange(n, dtype=np.int64)
    in_maps = []
    perms = []
    for c in range(n_cores):
        w_c = win[c * n:(c + 1) * n]
        q_c = q[c * n:(c + 1) * n]
        hi_q = q_c >= QSPLIT
        n2 = int(hi_q.sum())
        assert n2 <= tail, f"core {c}: {n2} high-query elements > tail {tail}"
        perm = np.concatenate([np.flatnonzero(~hi_q), np.flatnonzero(hi_q)])
        perms.append(perm)
        wp = w_c[perm]
        qp = q_c[perm]
        u1 = np.zeros((128, n), dtype=np.uint8)
        flat = wp * n + cols[:, None]
        u1[:V] = np.bincount(flat.ravel(), minlength=V * n).astype(
            np.uint8).reshape(V, n)
        low = np.flatnonzero(qp < QSPLIT)
        u1[V + qp[low], low] = 1
        ohq2 = np.zeros((V, tail), dtype=np.uint8)
        ohq2[qp[n - tail:], np.arange(tail)] = 1
        in_maps.append({
            "u1": u1.astype(F8), "ohq2": ohq2.astype(F8), "cblob": cblob,
        })
    return in_maps, perms


def _assemble(results, perms, n, tail):
    out = np.empty((len(perms) * n, H), dtype=np.float32)
    for c, perm in enumerate(perms):
        o1 = results[c]["out1"].astype(np.float32).T      # [n, H] permuted
        o2 = results[c]["out2"].astype(np.float32).T      # [tail, H]
        o1[n - tail:] = o2
        out[c * n:(c + 1) * n][perm] = o1
    return out


def kernel(seqs, query_tok, embed, W1, b1, W2, b2):
    from concourse.bass_utils import run_bass_kernel_spmd

    n = N_PER_CORE
    in_maps, perms = _host_prep(seqs, query_tok, embed, W1, b1, W2, b2,
                                NCORES, n, TAIL)
    nc = _get_program(n, CH, TS, TAIL)
    res = run_bass_kernel_spmd(nc, in_maps, core_ids=list(range(NCORES)))
    return _assemble(res.results, perms, n, TAIL)
